# revision 17
# baseline (speedup 1.0000x reference)
"""Multi-head self-attention kernel for 8 Trainium2 NeuronCores.

Problem: B=2, S=2048, D=1024, H=16 heads, head_dim=64 (fp32 in/out).

Sharding: tensor-parallel over heads. Core c owns heads {2c, 2c+1}, i.e.
output-feature range [c*128, (c+1)*128) of the Q/K/V projections and the
matching 128 contraction rows of the output projection. Each core computes a
full-shape partial of the output; the host sums the 8 partials and adds bo.

All device tensors are bf16 (inputs quantized on host; rel tolerance is
2e-2, measured end-to-end error ~4e-3). PSUM accumulation stays fp32.
bf16 halves HBM traffic vs fp32/f32r — the dominant HW cost since the 8
cores share chip HBM bandwidth — and unlocks the DVE 2x/4x wide modes.

Per-core device program:
  1. QT/KT/VT [128, 4096] = W_shard @ x.T  (x.T pre-transposed on host),
     PSUM fp32, bias added on DVE during the PSUM->SBUF eviction.
  2. V' [k, 65] per (batch, head): V with a ones column appended, built from
     VT by DVE 32x32 stream transposes (bf16 -> 2x DVE mode). The ones
     column makes the softmax denominator fall out of the ctx matmul free.
  3. Per (batch, head), per 128-wide k-chunk:
       scoresT [k=128, q=2048] = KT_chunk.T @ QT   (PSUM, 2 half-width tiles)
       PT = exp(0.125 * scoresT)                   (ScalarE, PSUM->SBUF bf16)
       ctx'T [65, q] += V'_chunk.T @ PT            (accumulating matmuls)
     Rows 0..63 of ctx'T are the unnormalized context, row 64 softmax sums.
  4. recip(sums) on DVE -> bf16 convert on Pool -> K=1 PE outer product
     broadcasts to 64 partitions in PSUM -> DVE copies to SBUF -> one DVE
     multiply writes scaled ctxT to a persistent [128, 4096] bf16 buffer.
  5. out_partial [t=128, 1024] = ctxT_chunk.T @ WoT_shard per t-chunk,
     evicted by DVE/ScalarE, DMA out as bf16.

Scheduling: batch-1 projections interleave into pair(0,0)'s chunk loop,
outproj(0) into pair(1,0); each pair's V'-build is hoisted into the middle
of the previous pair's chunk loop; softmax division work of pair N is
deferred past pair N+1's first chunk so the PE never stalls at a pair
boundary.
"""

import functools
import os
import sys

import numpy as np

for _p in ("/opt/trn_rl_repo", os.path.expanduser("~/.axon_site/_ro/trn_rl_repo")):
    if os.path.isdir(_p) and _p not in sys.path:
        sys.path.insert(0, _p)

import concourse.bass as bass
import concourse.tile as tile
from concourse import bacc
from concourse import mybir
from concourse.bass_utils import run_bass_kernel_spmd

F32 = mybir.dt.float32
BF16 = mybir.dt.bfloat16
AF = mybir.ActivationFunctionType

P = 128          # partitions / feature slice per core
B = 2            # batch
S = 2048         # sequence length
D = 1024         # embed dim
T = B * S        # total tokens
HD = 64          # head dim
KO = D // P      # contraction subtiles for the projections
NT = 8           # t-tiles for the projections
TW = 512         # projection t-tile width / matmul free dim
NKC = S // P     # 128-wide k-chunks per (batch, head)
NQS = S // TW    # 512-wide q-slices per (batch, head)
N_CORES = 8
SCALE = 1.0 / np.sqrt(np.float32(HD))  # 0.125


def _build_nc(n_reps: int = 1, phases: str = "full", dyn_reps: bool = False):
    nc = bacc.Bacc(target_bir_lowering=False, debug=False, num_devices=N_CORES)

    if dyn_reps:
        reps = nc.declare_dram_parameter("reps", [1, 1], mybir.dt.int32, isOutput=False)
    # xt4[tt, ki, ko, t] = x[tt*TW + t, ko*P + ki]; per-partition-contiguous DMA
    xt4 = nc.declare_dram_parameter("xt4", [NT, P, KO, TW], BF16, isOutput=False)
    wqT = nc.declare_dram_parameter("wqT", [P, KO, P], BF16, isOutput=False)
    wkT = nc.declare_dram_parameter("wkT", [P, KO, P], BF16, isOutput=False)
    wvT = nc.declare_dram_parameter("wvT", [P, KO, P], BF16, isOutput=False)
    woT = nc.declare_dram_parameter("woT", [P, D], BF16, isOutput=False)
    bq = nc.declare_dram_parameter("bq", [P, 1], F32, isOutput=False)
    bk = nc.declare_dram_parameter("bk", [P, 1], F32, isOutput=False)
    bv = nc.declare_dram_parameter("bv", [P, 1], F32, isOutput=False)
    out = nc.declare_dram_parameter("out", [T, D], BF16, isOutput=True)

    with tile.TileContext(nc) as tc:
        from contextlib import ExitStack

        with ExitStack() as ctx:
            singles = ctx.enter_context(tc.tile_pool(name="singles", bufs=1))
            qkv = ctx.enter_context(tc.tile_pool(name="qkv", bufs=1))
            xpool = ctx.enter_context(tc.tile_pool(name="xpool", bufs=3))
            ptpool = ctx.enter_context(tc.tile_pool(name="ptpool", bufs=6))
            vpool = ctx.enter_context(tc.tile_pool(name="vpool", bufs=2))
            rpool = ctx.enter_context(tc.tile_pool(name="rpool", bufs=2))
            opool = ctx.enter_context(tc.tile_pool(name="opool", bufs=6))
            sppool = ctx.enter_context(
                tc.tile_pool(name="sppool", bufs=2, space="PSUM")
            )
            pbank = ctx.enter_context(tc.tile_pool(name="pbank", bufs=4, space="PSUM"))
            pools = (singles, qkv, xpool, ptpool, vpool, rpool, opool, sppool,
                     pbank, xt4, wqT, wkT, wvT, woT, bq, bk, bv, out)

            from contextlib import nullcontext

            if dyn_reps:
                reps_sb = singles.tile([1, 1], mybir.dt.int32, name="reps_sb")
                nc.sync.dma_start(out=reps_sb[:], in_=reps[:])
                reps_val = nc.values_load(reps_sb[:], min_val=0, max_val=1 << 20)
                rep_loop = tc.For_i(0, reps_val, 1)
            elif n_reps > 1:
                rep_loop = tc.For_i(0, n_reps, 1)
            else:
                rep_loop = nullcontext()
            with rep_loop:
                _kernel_body(nc, tc, pools, phases)

    nc.finalize()
    return nc


def _emit_recips(nc, rpool, ctx_ps, bb, h):
    # DVE reciprocal of the softmax sums, converted to bf16 per q-slice so
    # rs_r becomes available incrementally and the deferred PE broadcast of
    # the next pair never waits long on DVE.
    rs = rpool.tile([1, S], F32, tag="rs", name=f"rs_{bb}_{h}")
    rs_r = rpool.tile([1, S], BF16, tag="rs_r", name=f"rs_r_{bb}_{h}")
    for qs in range(NQS):
        nc.vector.reciprocal(
            rs[:, qs * TW : (qs + 1) * TW], ctx_ps[qs][HD : HD + 1, :]
        )
        nc.vector.tensor_copy(
            rs_r[:, qs * TW : (qs + 1) * TW], rs[:, qs * TW : (qs + 1) * TW]
        )
    return rs_r


def _emit_division(nc, rpool, sppool, ones_r, ctx_ps, CX, pb, base, bb, h, rs_r):
    rb = rpool.tile([HD, S], BF16, tag="rb", name=f"rb_{bb}_{h}")
    for half in range(2):
        rbp = sppool.tile([HD, 2 * TW], F32, tag="sp", name=f"rbp_{bb}_{h}_{half}")
        for j in range(2):
            qs = half * 2 + j
            nc.tensor.matmul(
                rbp[:, j * TW : (j + 1) * TW],
                ones_r[:],
                rs_r[:, qs * TW : (qs + 1) * TW],
                start=True,
                stop=True,
            )
            # PSUM -> SBUF so the scale mul has one PSUM operand
            nc.vector.tensor_copy(
                rb[:, qs * TW : (qs + 1) * TW], rbp[:, j * TW : (j + 1) * TW]
            )
            nc.vector.tensor_mul(
                out=CX[pb : pb + HD, base + qs * TW : base + (qs + 1) * TW],
                in0=ctx_ps[qs][0:HD, :],
                in1=rb[:, qs * TW : (qs + 1) * TW],
            )


def _kernel_body(nc, tc, pools, phases="full"):
    (singles, qkv, xpool, ptpool, vpool, rpool, opool, sppool, pbank,
     xt4, wqT, wkT, wvT, woT, bq, bk, bv, out) = pools
    if True:
        if True:
            # ---- weights / biases to SBUF ----
            # DMA issue order is tuned so the first projection matmuls are
            # gated only on xt(tt=0) + wq, each split across many HWDGE
            # queues; wk/wv/biases follow, wo is deferred to just before the
            # output projection.
            def load_xt(tt, nsplit=2):
                xt = xpool.tile([P, KO, TW], BF16, tag="xt", name=f"xt_{tt}")
                # split so the first ko chunks (and the first matmuls gated
                # on them) land early; tt=0 is split finest since it gates
                # the whole pipeline
                step = KO // nsplit
                for hf in range(nsplit):
                    nc.sync.dma_start(
                        out=xt[:, step * hf : step * (hf + 1)],
                        in_=xt4[:][tt, :, step * hf : step * (hf + 1)],
                    )
                return xt

            # wq + x go through the SP DMA queue; wk/wv/biases issue in
            # parallel from the DVE and ACT sequencers (the SP sequencer
            # takes 565ns per dma_start, serially -- putting everything on
            # SP would gate the first K/V matmuls ~10us out)
            w_sbs = [
                singles.tile([P, KO, P], BF16, tag=f"{name}_sb",
                             name=f"{name}_sb")
                for name in ("wq", "wk", "wv")
            ]
            for hf in range(2):
                nc.sync.dma_start(
                    out=w_sbs[0][:, 4 * hf : 4 * hf + 4],
                    in_=wqT[:][:, 4 * hf : 4 * hf + 4],
                )
            xt0 = load_xt(0, nsplit=4)
            nc.scalar.dma_start(out=w_sbs[1][:], in_=wkT[:])
            nc.scalar.dma_start(out=w_sbs[2][:], in_=wvT[:])
            b_sbs = []
            for name, bdram in (("bq", bq), ("bk", bk), ("bv", bv)):
                b_sb = singles.tile([P, 1], F32, tag=f"{name}_sb")
                nc.scalar.dma_start(out=b_sb[:], in_=bdram[:])
                b_sbs.append(b_sb)
            wo_sb = singles.tile([P, D], BF16, tag="wo_sb")

            def load_wo():
                for hf in range(2):
                    nc.sync.dma_start(
                        out=wo_sb[:, hf * TW : (hf + 1) * TW],
                        in_=woT[:][:, hf * TW : (hf + 1) * TW],
                    )

            # ones row for PE-broadcast of softmax reciprocals (K=1 matmul)
            ones_r = singles.tile([1, HD], BF16, tag="ones_r")
            nc.vector.memset(ones_r[:], 1.0)

            # ---- persistent activations ----
            QT = qkv.tile([P, T], BF16, tag="QT")
            KT = qkv.tile([P, T], BF16, tag="KT")
            VT = qkv.tile([P, T], BF16, tag="VT")
            CX = qkv.tile([P, T], BF16, tag="CX")  # scaled ctxT, both heads

            # ---- projections: QT/KT/VT[f, t] = sum_d W[d, f] * xT[d, t] ----

            def proj_tile(tt, xt=None):
                if xt is None:
                    xt = load_xt(tt)
                for w_sb, b_sb, dst in zip(w_sbs, b_sbs, (QT, KT, VT), strict=True):
                    ps = sppool.tile([P, TW], F32, tag="sp")
                    for ko in range(KO):
                        nc.tensor.matmul(
                            ps[:],
                            w_sb[:, ko],
                            xt[:, ko],
                            start=(ko == 0),
                            stop=(ko == KO - 1),
                        )
                    nc.vector.tensor_scalar_add(
                        dst[:, tt * TW : (tt + 1) * TW], ps[:], b_sb[:]
                    )
                    yield

            # batch-0 projections first
            for tt in range(NT // 2):
                for _ in proj_tile(tt, xt0 if tt == 0 else None):
                    pass

            # ---- attention for one (batch, head) pair (generator:
            #      yields before and after each k-chunk) ----
            def vbuild(bb: int, h: int):
                # V' [k-part, kc, 65]: V plus a ones column, via DVE 32x32
                # stream transposes of VT. bf16 throughout: the transposes
                # run in the DVE 2x mode and no convert-copy is needed.
                base = bb * S
                pb = h * HD
                vp = vpool.tile(
                    [P, NKC, HD + 1], BF16, tag="vp", name=f"vp_{bb}_{h}"
                )
                nc.vector.memset(vp[:, :, HD], 1.0)
                for a in range(2):
                    src = VT[pb + 32 * a : pb + 32 * (a + 1), base : base + S]
                    src = src.rearrange("p (kc r) -> p kc r", r=P)
                    for b2 in range(4):
                        nc.vector.transpose(
                            vp[32 * b2 : 32 * (b2 + 1), :, 32 * a : 32 * (a + 1)],
                            src[:, :, 32 * b2 : 32 * (b2 + 1)],
                        )
                return vp

            def pair_attn(bb: int, h: int, vp_of):
                base = bb * S       # token offset of this batch
                pb = h * HD         # partition offset of this head in QT/KT/VT
                yield
                vp = vp_of()

                ctx_ps = [
                    pbank.tile([HD + 1, TW], F32, tag="pb", name=f"ctx_ps_{bb}_{h}_{i}")
                    for i in range(NQS)
                ]
                # Two half-width scores tiles per k-chunk, double-buffered
                # (sppool bufs=2), so PE can fill one half while ScalarE
                # exps the other. The ctx matmuls for chunk kc are emitted
                # LAG chunks later (software pipeline): the first ctx matmul
                # of this pair then sits far enough back in the PE queue
                # that the previous pair's deferred softmax-division -- which
                # must release the ctx PSUM banks -- never stalls the PE.
                HW2 = S // 2  # 1024 q per half
                LAG = 2
                pts = {}

                def emit_scores(kc):
                    kt_chunk = KT[pb : pb + HD, base + kc * P : base + (kc + 1) * P]
                    for half in range(2):
                        sp = sppool.tile(
                            [P, HW2], F32, tag="sp", name=f"sp_{bb}_{h}_{kc}_{half}"
                        )
                        q0 = half * HW2
                        for qs in range(2):
                            nc.tensor.matmul(
                                sp[:, qs * TW : (qs + 1) * TW],
                                kt_chunk,
                                QT[
                                    pb : pb + HD,
                                    base + q0 + qs * TW : base + q0 + (qs + 1) * TW,
                                ],
                                start=True,
                                stop=True,
                            )
                        pt = ptpool.tile(
                            [P, HW2], BF16, tag="pt", name=f"pt_{bb}_{h}_{kc}_{half}"
                        )
                        nc.scalar.activation(pt[:], sp[:], AF.Exp, scale=float(SCALE))
                        pts[(kc, half)] = pt

                def emit_ctx(kc):
                    vchunk = vp[:, kc]
                    for half in range(2):
                        pt = pts.pop((kc, half))
                        for qs in range(2):
                            nc.tensor.matmul(
                                ctx_ps[half * 2 + qs][:],
                                vchunk,
                                pt[:, qs * TW : (qs + 1) * TW],
                                start=(kc == 0),
                                stop=(kc == NKC - 1),
                            )

                for kc in range(NKC):
                    emit_scores(kc)
                    if kc >= LAG:
                        emit_ctx(kc - LAG)
                    yield
                for kc in range(NKC - LAG, NKC):
                    emit_ctx(kc)

                # softmax denominators: reciprocal (DVE), broadcast to 64
                # partitions via a K=1 PE outer product into free scores-pool
                # PSUM, then scale ctx rows. Emitted via a deferred closure so
                # the PE queue is not blocked at the pair boundary.
                rs_r = _emit_recips(nc, rpool, ctx_ps, bb, h)

                def division():
                    _emit_division(
                        nc, rpool, sppool, ones_r, ctx_ps, CX, pb, base, bb, h,
                        rs_r,
                    )

                yield division

            # ---- output projection for one batch (generator) ----
            # copies="dve" keeps the PSUM evacuation entirely on DVE so ACT
            # stays exp-only while interleaved into an attention pair;
            # copies="split" shares DVE/ACT for the un-interleaved tail.
            def outproj(bb: int, ps_pool=None, ps_tag="sp", copies="split"):
                ps_pool = ps_pool or sppool
                for tci in range(S // P):
                    tg = bb * (S // P) + tci
                    ot = opool.tile([P, D], BF16, tag="ot")
                    for half in range(2):
                        ps = ps_pool.tile([P, TW], F32, tag=ps_tag)
                        nc.tensor.matmul(
                            ps[:],
                            CX[:, tg * P : (tg + 1) * P],
                            wo_sb[:, half * TW : (half + 1) * TW],
                            start=True,
                            stop=True,
                        )
                        if half == 0 or copies == "dve":
                            nc.vector.tensor_copy(
                                ot[:, half * TW : (half + 1) * TW], ps[:]
                            )
                        else:
                            nc.scalar.copy(
                                ot[:, half * TW : (half + 1) * TW], ps[:]
                            )
                    nc.sync.dma_start(
                        out=out[:][tg * P : (tg + 1) * P, :],
                        in_=ot[:],
                    )
                    yield

            if phases == "proj":
                for tt in range(NT // 2, NT):
                    for _ in proj_tile(tt):
                        pass
                for i, t_ in enumerate((QT, KT, VT)):
                    for j in range(4):
                        nc.sync.dma_start(
                            out=out[:][(4 * i + j) * P : (4 * i + j + 1) * P, :],
                            in_=t_[:, j * D : (j + 1) * D],
                        )
                return

            import itertools

            def run_pair(gen, prev_div=None, interleave=None, per_step=0,
                         interleave_from=1, actions=None, div_at=3):
                # Drive a pair generator. Yields are: pre-chunk slot, one per
                # k-chunk, then the pair's deferred division closure. The
                # previous pair's division is emitted after this pair's
                # chunk (div_at - 2): late enough that its PE broadcasts
                # don't stall the PE queue head waiting on rs_r, but still
                # before this pair's first ctx matmul (emitted at chunk LAG,
                # i.e. yield LAG+2) which needs the PSUM banks the division
                # releases -- emitting it later would deadlock the PE queue.
                division = None
                n = 0
                for item in gen:
                    if callable(item):
                        division = item
                        continue
                    n += 1
                    if prev_div is not None and n == div_at:
                        prev_div()
                        prev_div = None
                    if interleave is not None and n >= interleave_from:
                        for _ in range(per_step):
                            next(interleave, None)
                    if actions and n in actions:
                        actions[n]()
                if prev_div is not None:
                    prev_div()
                return division

            # V'-builds for pair k+1 are hoisted into pair k's chunk loop
            # (DVE is idle mid-pair) so the pair boundary never waits on the
            # stream transposes.
            vps = {}

            def vb_action(bb, h):
                return lambda: vps.__setitem__((bb, h), vbuild(bb, h))

            def vp_of(bb, h):
                return lambda: vps[(bb, h)]

            # batch-1 projections: 8 W-steps paced one-per-chunk into
            # pair(0,0), the remaining 4 into pair(0,1). Pacing the extra PE
            # work evenly keeps the PE ahead of ACT's exp stream in every
            # pair, so no exp backlog accumulates to stall pair boundaries.
            proj_steps = itertools.chain(
                *[proj_tile(tt) for tt in range(NT // 2, NT)]
            )
            vps[(0, 0)] = vbuild(0, 0)
            div00 = run_pair(
                pair_attn(0, 0, vp_of(0, 0)), None,
                itertools.islice(proj_steps, 6), per_step=1,
                interleave_from=2, actions={9: vb_action(0, 1)},
            )
            div01 = run_pair(
                pair_attn(0, 1, vp_of(0, 1)), prev_div=div00,
                interleave=proj_steps, per_step=1, interleave_from=2,
                actions={8: vb_action(1, 0)},
            )
            if phases == "attn2":
                div01()
                nc.sync.dma_start(out=out[:][0:P, :], in_=CX[:, 0:D])
                return
            load_wo()
            op0 = outproj(0, copies="dve")
            div10 = run_pair(
                pair_attn(1, 0, vp_of(1, 0)), prev_div=div01,
                interleave=itertools.islice(op0, 8), per_step=1,
                interleave_from=3, actions={12: vb_action(1, 1)},
            )
            div11 = run_pair(
                pair_attn(1, 1, vp_of(1, 1)), prev_div=div10,
                interleave=op0, per_step=1, interleave_from=2,
            )
            div11()
            for _ in outproj(1, ps_pool=pbank, ps_tag="pb"):
                pass


@functools.lru_cache(maxsize=8)
def _get_nc(n_reps: int = 1, phases: str = "full", dyn_reps: bool = False):
    return _build_nc(n_reps, phases, dyn_reps)


def _to_bf16(a):
    import ml_dtypes

    return np.asarray(a, dtype=np.float32).astype(ml_dtypes.bfloat16)


def _shard_inputs(x, Wq, bq, Wk, bk, Wv, bv, Wo, bo):
    x = _to_bf16(x).reshape(T, D)
    # xt4[tt, ki, ko, t] = x[tt*TW + t, ko*P + ki]
    xt4 = np.ascontiguousarray(
        x.reshape(NT, TW, KO, P).transpose(0, 3, 2, 1)
    )
    Wq = _to_bf16(Wq)
    Wk = _to_bf16(Wk)
    Wv = _to_bf16(Wv)
    Wo = _to_bf16(Wo)
    bq = np.asarray(bq, dtype=np.float32)
    bk = np.asarray(bk, dtype=np.float32)
    bv = np.asarray(bv, dtype=np.float32)

    def wtile(W, sl):
        # [ki, ko, f] = W[c*P + f, ko*P + ki]
        return np.ascontiguousarray(
            W[sl, :].reshape(P, KO, P).transpose(2, 1, 0)
        )

    in_maps = []
    for c in range(N_CORES):
        sl = slice(c * P, (c + 1) * P)
        in_maps.append(
            {
                "xt4": xt4,
                "wqT": wtile(Wq, sl),
                "wkT": wtile(Wk, sl),
                "wvT": wtile(Wv, sl),
                "woT": np.ascontiguousarray(Wo[:, sl].T),
                "bq": np.ascontiguousarray(bq[sl].reshape(P, 1)),
                "bk": np.ascontiguousarray(bk[sl].reshape(P, 1)),
                "bv": np.ascontiguousarray(bv[sl].reshape(P, 1)),
            }
        )
    return in_maps


def kernel(x, Wq, bq, Wk, bk, Wv, bv, Wo, bo, **run_kwargs):
    nc = _get_nc()
    in_maps = _shard_inputs(x, Wq, bq, Wk, bk, Wv, bv, Wo, bo)
    last_exc = None
    for _attempt in range(3):
        try:
            res = run_bass_kernel_spmd(
                nc, in_maps, core_ids=list(range(N_CORES)), **run_kwargs
            )
            break
        except Exception as exc:  # transient device errors: retry
            last_exc = exc
            import time as _time

            _time.sleep(3.0)
            # a wedged PJRT client never recovers in-process; force a fresh
            # backend connection so the retry sees recovered devices
            try:
                import jax as _jax

                _jax.clear_caches()
                _jax.extend.backend.clear_backends()
            except Exception:
                pass
    else:
        raise last_exc
    partials = [r["out"] for r in res.results]
    acc = np.add.reduce([np.asarray(p, dtype=np.float32) for p in partials], axis=0)
    acc = acc + np.asarray(bo, dtype=np.float32)[None, :]
    if run_kwargs:
        kernel.last_results = res
    return acc.reshape(B, S, D).astype(np.float32)


# revision 21
# speedup vs baseline: 1.0983x; 1.0983x over previous
"""Multi-head self-attention kernel for 8 Trainium2 NeuronCores.

Problem: B=2, S=2048, D=1024, H=16 heads, head_dim=64 (fp32 in/out).

Sharding: tensor-parallel over heads. Core c owns heads {2c, 2c+1}, i.e.
output-feature range [c*128, (c+1)*128) of the Q/K/V projections and the
matching 128 contraction rows of the output projection. Each core computes a
full-shape partial of the output; the host sums the 8 partials and adds bo.

All device tensors are bf16 (inputs quantized on host; rel tolerance is
2e-2, measured end-to-end error ~4e-3). PSUM accumulation stays fp32.
bf16 halves HBM traffic vs fp32/f32r — the dominant HW cost since the 8
cores share chip HBM bandwidth — and unlocks the DVE 2x/4x wide modes.

Per-core device program:
  1. QT/KT/VT [128, 4096] = W_shard @ x.T  (x.T pre-transposed on host),
     PSUM fp32, bias added on DVE during the PSUM->SBUF eviction.
  2. V' [k, 65] per (batch, head): V with a ones column appended, built from
     VT by DVE 32x32 stream transposes (bf16 -> 2x DVE mode). The ones
     column makes the softmax denominator fall out of the ctx matmul free.
  3. Per (batch, head), per 128-wide k-chunk:
       scoresT [k=128, q=2048] = KT_chunk.T @ QT   (PSUM, 2 half-width tiles)
       PT = exp(0.125 * scoresT)                   (ScalarE, PSUM->SBUF bf16)
       ctx'T [65, q] += V'_chunk.T @ PT            (accumulating matmuls)
     Rows 0..63 of ctx'T are the unnormalized context, row 64 softmax sums.
  4. recip(sums) on DVE -> bf16 convert on Pool -> K=1 PE outer product
     broadcasts to 64 partitions in PSUM -> DVE copies to SBUF -> one DVE
     multiply writes scaled ctxT to a persistent [128, 4096] bf16 buffer.
  5. out_partial [t=128, 1024] = ctxT_chunk.T @ WoT_shard per t-chunk,
     evicted by DVE/ScalarE, DMA out as bf16.

Scheduling: batch-1 projections interleave into pair(0,0)'s chunk loop,
outproj(0) into pair(1,0); each pair's V'-build is hoisted into the middle
of the previous pair's chunk loop; softmax division work of pair N is
deferred past pair N+1's first chunk so the PE never stalls at a pair
boundary.
"""

import functools
import os
import sys

import numpy as np

for _p in ("/opt/trn_rl_repo", os.path.expanduser("~/.axon_site/_ro/trn_rl_repo")):
    if os.path.isdir(_p) and _p not in sys.path:
        sys.path.insert(0, _p)

import concourse.bass as bass
import concourse.tile as tile
from concourse import bacc
from concourse import mybir
from concourse.bass_utils import run_bass_kernel_spmd

F32 = mybir.dt.float32
BF16 = mybir.dt.bfloat16
AF = mybir.ActivationFunctionType

P = 128          # partitions / feature slice per core
B = 2            # batch
S = 2048         # sequence length
D = 1024         # embed dim
T = B * S        # total tokens
HD = 64          # head dim
KO = D // P      # contraction subtiles for the projections
NT = 8           # t-tiles for the projections
TW = 512         # projection t-tile width / matmul free dim
NKC = S // P     # 128-wide k-chunks per (batch, head)
NQS = S // TW    # 512-wide q-slices per (batch, head)
N_CORES = 8
SCALE = 1.0 / np.sqrt(np.float32(HD))  # 0.125


def _build_nc(n_reps: int = 1, phases: str = "full", dyn_reps: bool = False):
    nc = bacc.Bacc(target_bir_lowering=False, debug=False, num_devices=N_CORES)

    if dyn_reps:
        reps = nc.declare_dram_parameter("reps", [1, 1], mybir.dt.int32, isOutput=False)
    # xt4[tt, ki, ko, t] = x[tt*TW + t, ko*P + ki]; per-partition-contiguous DMA
    xt4 = nc.declare_dram_parameter("xt4", [NT, P, KO, TW], BF16, isOutput=False)
    wqT = nc.declare_dram_parameter("wqT", [P, KO, P], BF16, isOutput=False)
    wkT = nc.declare_dram_parameter("wkT", [P, KO, P], BF16, isOutput=False)
    wvT = nc.declare_dram_parameter("wvT", [P, KO, P], BF16, isOutput=False)
    woT = nc.declare_dram_parameter("woT", [P, D], BF16, isOutput=False)
    bq = nc.declare_dram_parameter("bq", [P, 1], F32, isOutput=False)
    bk = nc.declare_dram_parameter("bk", [P, 1], F32, isOutput=False)
    bv = nc.declare_dram_parameter("bv", [P, 1], F32, isOutput=False)
    out = nc.declare_dram_parameter("out", [T, D], BF16, isOutput=True)

    with tile.TileContext(nc) as tc:
        from contextlib import ExitStack

        with ExitStack() as ctx:
            singles = ctx.enter_context(tc.tile_pool(name="singles", bufs=1))
            qkv = ctx.enter_context(tc.tile_pool(name="qkv", bufs=1))
            xpool = ctx.enter_context(tc.tile_pool(name="xpool", bufs=3))
            ptpool = ctx.enter_context(tc.tile_pool(name="ptpool", bufs=6))
            vpool = ctx.enter_context(tc.tile_pool(name="vpool", bufs=2))
            rpool = ctx.enter_context(tc.tile_pool(name="rpool", bufs=2))
            opool = ctx.enter_context(tc.tile_pool(name="opool", bufs=6))
            sppool = ctx.enter_context(
                tc.tile_pool(name="sppool", bufs=2, space="PSUM")
            )
            pbank = ctx.enter_context(tc.tile_pool(name="pbank", bufs=4, space="PSUM"))
            pools = (singles, qkv, xpool, ptpool, vpool, rpool, opool, sppool,
                     pbank, xt4, wqT, wkT, wvT, woT, bq, bk, bv, out)

            from contextlib import nullcontext

            if dyn_reps:
                reps_sb = singles.tile([1, 1], mybir.dt.int32, name="reps_sb")
                nc.sync.dma_start(out=reps_sb[:], in_=reps[:])
                reps_val = nc.values_load(reps_sb[:], min_val=0, max_val=1 << 20)
                rep_loop = tc.For_i(0, reps_val, 1)
            elif n_reps > 1:
                rep_loop = tc.For_i(0, n_reps, 1)
            else:
                rep_loop = nullcontext()
            with rep_loop:
                _kernel_body(nc, tc, pools, phases)

    nc.finalize()
    return nc


def _emit_recips(nc, rpool, ctx_ps, bb, h):
    # DVE reciprocal of the softmax sums, converted to bf16 per q-slice so
    # rs_r becomes available incrementally and the deferred PE broadcast of
    # the next pair never waits long on DVE.
    rs = rpool.tile([1, S], F32, tag="rs", name=f"rs_{bb}_{h}")
    rs_r = rpool.tile([1, S], BF16, tag="rs_r", name=f"rs_r_{bb}_{h}")
    for qs in range(NQS):
        nc.vector.reciprocal(
            rs[:, qs * TW : (qs + 1) * TW], ctx_ps[qs][HD : HD + 1, :]
        )
        nc.vector.tensor_copy(
            rs_r[:, qs * TW : (qs + 1) * TW], rs[:, qs * TW : (qs + 1) * TW]
        )
    return rs_r


def _emit_division(nc, rpool, sppool, ones_r, ctx_ps, CX, pb, base, bb, h, rs_r,
                   after_qs=None, rb_engine="act"):
    rb = rpool.tile([HD, S], BF16, tag="rb", name=f"rb_{bb}_{h}")
    for half in range(2):
        rbp = sppool.tile([HD, 2 * TW], F32, tag="sp", name=f"rbp_{bb}_{h}_{half}")
        for j in range(2):
            qs = half * 2 + j
            nc.tensor.matmul(
                rbp[:, j * TW : (j + 1) * TW],
                ones_r[:],
                rs_r[:, qs * TW : (qs + 1) * TW],
                start=True,
                stop=True,
            )
            # PSUM -> SBUF so the scale mul has one PSUM operand.
            # rb_engine picks the copy engine: DVE when emitted mid-pair
            # (ACT is busy streaming exps there), ACT for the tail division
            # (where DVE carries the outproj evacuations).
            if rb_engine == "dve":
                nc.vector.tensor_copy(
                    rb[:, qs * TW : (qs + 1) * TW], rbp[:, j * TW : (j + 1) * TW]
                )
            else:
                nc.scalar.copy(
                    rb[:, qs * TW : (qs + 1) * TW], rbp[:, j * TW : (j + 1) * TW]
                )
            nc.vector.tensor_mul(
                out=CX[pb : pb + HD, base + qs * TW : base + (qs + 1) * TW],
                in0=ctx_ps[qs][0:HD, :],
                in1=rb[:, qs * TW : (qs + 1) * TW],
            )
            if after_qs is not None:
                after_qs(qs)


def _kernel_body(nc, tc, pools, phases="full"):
    (singles, qkv, xpool, ptpool, vpool, rpool, opool, sppool, pbank,
     xt4, wqT, wkT, wvT, woT, bq, bk, bv, out) = pools
    if True:
        if True:
            # ---- weights / biases to SBUF ----
            # DMA issue order is tuned so the first projection matmuls are
            # gated only on xt(tt=0) + wq, each split across many HWDGE
            # queues; wk/wv/biases follow, wo is deferred to just before the
            # output projection.
            def load_xt(tt, nsplit=2):
                xt = xpool.tile([P, KO, TW], BF16, tag="xt", name=f"xt_{tt}")
                # split so the first ko chunks (and the first matmuls gated
                # on them) land early; tt=0 is split finest since it gates
                # the whole pipeline
                step = KO // nsplit
                for hf in range(nsplit):
                    nc.sync.dma_start(
                        out=xt[:, step * hf : step * (hf + 1)],
                        in_=xt4[:][tt, :, step * hf : step * (hf + 1)],
                    )
                return xt

            # wq + x go through the SP DMA queue; wk/wv/biases issue in
            # parallel from the DVE and ACT sequencers (the SP sequencer
            # takes 565ns per dma_start, serially -- putting everything on
            # SP would gate the first K/V matmuls ~10us out)
            w_sbs = [
                singles.tile([P, KO, P], BF16, tag=f"{name}_sb",
                             name=f"{name}_sb")
                for name in ("wq", "wk", "wv")
            ]
            for hf in range(2):
                nc.sync.dma_start(
                    out=w_sbs[0][:, 4 * hf : 4 * hf + 4],
                    in_=wqT[:][:, 4 * hf : 4 * hf + 4],
                )
            xt0 = load_xt(0, nsplit=4)
            nc.scalar.dma_start(out=w_sbs[1][:], in_=wkT[:])
            nc.scalar.dma_start(out=w_sbs[2][:], in_=wvT[:])
            b_sbs = []
            for name, bdram in (("bq", bq), ("bk", bk), ("bv", bv)):
                b_sb = singles.tile([P, 1], F32, tag=f"{name}_sb")
                nc.scalar.dma_start(out=b_sb[:], in_=bdram[:])
                b_sbs.append(b_sb)
            wo_sb = singles.tile([P, D], BF16, tag="wo_sb")

            def load_wo():
                for hf in range(2):
                    nc.sync.dma_start(
                        out=wo_sb[:, hf * TW : (hf + 1) * TW],
                        in_=woT[:][:, hf * TW : (hf + 1) * TW],
                    )

            # ones row for PE-broadcast of softmax reciprocals (K=1 matmul)
            ones_r = singles.tile([1, HD], BF16, tag="ones_r")
            nc.vector.memset(ones_r[:], 1.0)

            # ---- persistent activations ----
            QT = qkv.tile([P, T], BF16, tag="QT")
            KT = qkv.tile([P, T], BF16, tag="KT")
            VT = qkv.tile([P, T], BF16, tag="VT")
            CX = qkv.tile([P, T], BF16, tag="CX")  # scaled ctxT, both heads

            # ---- projections: QT/KT/VT[f, t] = sum_d W[d, f] * xT[d, t] ----

            def proj_tile(tt, xt=None):
                if xt is None:
                    xt = load_xt(tt)
                for w_sb, b_sb, dst in zip(w_sbs, b_sbs, (QT, KT, VT), strict=True):
                    ps = sppool.tile([P, TW], F32, tag="sp")
                    for ko in range(KO):
                        nc.tensor.matmul(
                            ps[:],
                            w_sb[:, ko],
                            xt[:, ko],
                            start=(ko == 0),
                            stop=(ko == KO - 1),
                        )
                    nc.vector.tensor_scalar_add(
                        dst[:, tt * TW : (tt + 1) * TW], ps[:], b_sb[:]
                    )
                    yield

            # batch-0 projections first
            for tt in range(NT // 2):
                for _ in proj_tile(tt, xt0 if tt == 0 else None):
                    pass

            # ---- attention for one (batch, head) pair (generator:
            #      yields before and after each k-chunk) ----
            def vbuild(bb: int, h: int):
                # V' [k-part, kc, 65]: V plus a ones column, via DVE 32x32
                # stream transposes of VT. bf16 throughout: the transposes
                # run in the DVE 2x mode and no convert-copy is needed.
                base = bb * S
                pb = h * HD
                vp = vpool.tile(
                    [P, NKC, HD + 1], BF16, tag="vp", name=f"vp_{bb}_{h}"
                )
                nc.vector.memset(vp[:, :, HD], 1.0)
                for a in range(2):
                    src = VT[pb + 32 * a : pb + 32 * (a + 1), base : base + S]
                    src = src.rearrange("p (kc r) -> p kc r", r=P)
                    for b2 in range(4):
                        nc.vector.transpose(
                            vp[32 * b2 : 32 * (b2 + 1), :, 32 * a : 32 * (a + 1)],
                            src[:, :, 32 * b2 : 32 * (b2 + 1)],
                        )
                return vp

            def pair_attn(bb: int, h: int, vp_of):
                base = bb * S       # token offset of this batch
                pb = h * HD         # partition offset of this head in QT/KT/VT
                yield
                vp = vp_of()

                ctx_ps = [
                    pbank.tile([HD + 1, TW], F32, tag="pb", name=f"ctx_ps_{bb}_{h}_{i}")
                    for i in range(NQS)
                ]
                # Two half-width scores tiles per k-chunk, double-buffered
                # (sppool bufs=2), so PE can fill one half while ScalarE
                # exps the other. The ctx matmuls for chunk kc are emitted
                # LAG chunks later (software pipeline): the first ctx matmul
                # of this pair then sits far enough back in the PE queue
                # that the previous pair's deferred softmax-division -- which
                # must release the ctx PSUM banks -- never stalls the PE.
                HW2 = S // 2  # 1024 q per half
                LAG = 2
                pts = {}

                def emit_scores(kc):
                    kt_chunk = KT[pb : pb + HD, base + kc * P : base + (kc + 1) * P]
                    for half in range(2):
                        sp = sppool.tile(
                            [P, HW2], F32, tag="sp", name=f"sp_{bb}_{h}_{kc}_{half}"
                        )
                        q0 = half * HW2
                        for qs in range(2):
                            nc.tensor.matmul(
                                sp[:, qs * TW : (qs + 1) * TW],
                                kt_chunk,
                                QT[
                                    pb : pb + HD,
                                    base + q0 + qs * TW : base + q0 + (qs + 1) * TW,
                                ],
                                start=True,
                                stop=True,
                            )
                        pt = ptpool.tile(
                            [P, HW2], BF16, tag="pt", name=f"pt_{bb}_{h}_{kc}_{half}"
                        )
                        nc.scalar.activation(pt[:], sp[:], AF.Exp, scale=float(SCALE))
                        pts[(kc, half)] = pt

                def emit_ctx(kc):
                    vchunk = vp[:, kc]
                    for half in range(2):
                        pt = pts.pop((kc, half))
                        for qs in range(2):
                            nc.tensor.matmul(
                                ctx_ps[half * 2 + qs][:],
                                vchunk,
                                pt[:, qs * TW : (qs + 1) * TW],
                                start=(kc == 0),
                                stop=(kc == NKC - 1),
                            )

                for kc in range(NKC):
                    emit_scores(kc)
                    if kc >= LAG:
                        emit_ctx(kc - LAG)
                    yield
                for kc in range(NKC - LAG, NKC):
                    emit_ctx(kc)

                # softmax denominators: reciprocal (DVE), broadcast to 64
                # partitions via a K=1 PE outer product into free scores-pool
                # PSUM, then scale ctx rows. Emitted via a deferred closure so
                # the PE queue is not blocked at the pair boundary.
                rs_r = _emit_recips(nc, rpool, ctx_ps, bb, h)

                def division(after_qs=None, rb_engine="act"):
                    _emit_division(
                        nc, rpool, sppool, ones_r, ctx_ps, CX, pb, base, bb, h,
                        rs_r, after_qs=after_qs, rb_engine=rb_engine,
                    )

                yield division

            # ---- output projection for one batch (generator) ----
            # copies="dve" keeps the PSUM evacuation entirely on DVE so ACT
            # stays exp-only while interleaved into an attention pair;
            # copies="split" shares DVE/ACT for the un-interleaved tail.
            def outproj(bb: int, ps_pool=None, ps_tag="sp", copies="split"):
                ps_pool = ps_pool or sppool
                for tci in range(S // P):
                    tg = bb * (S // P) + tci
                    ot = opool.tile([P, D], BF16, tag="ot")
                    for half in range(2):
                        ps = ps_pool.tile([P, TW], F32, tag=ps_tag)
                        nc.tensor.matmul(
                            ps[:],
                            CX[:, tg * P : (tg + 1) * P],
                            wo_sb[:, half * TW : (half + 1) * TW],
                            start=True,
                            stop=True,
                        )
                        if half == 0 or copies == "dve":
                            nc.vector.tensor_copy(
                                ot[:, half * TW : (half + 1) * TW], ps[:]
                            )
                        else:
                            nc.scalar.copy(
                                ot[:, half * TW : (half + 1) * TW], ps[:]
                            )
                    nc.sync.dma_start(
                        out=out[:][tg * P : (tg + 1) * P, :],
                        in_=ot[:],
                    )
                    yield

            if phases == "proj":
                for tt in range(NT // 2, NT):
                    for _ in proj_tile(tt):
                        pass
                for i, t_ in enumerate((QT, KT, VT)):
                    for j in range(4):
                        nc.sync.dma_start(
                            out=out[:][(4 * i + j) * P : (4 * i + j + 1) * P, :],
                            in_=t_[:, j * D : (j + 1) * D],
                        )
                return

            import itertools

            def run_pair(gen, prev_div=None, interleave=None, per_step=0,
                         interleave_from=1, actions=None, div_at=3, every=2):
                # Drive a pair generator. Yields are: pre-chunk slot, one per
                # k-chunk, then the pair's deferred division closure. The
                # previous pair's division is emitted after this pair's
                # chunk (div_at - 2): late enough that its PE broadcasts
                # don't stall the PE queue head waiting on rs_r, but still
                # before this pair's first ctx matmul (emitted at chunk LAG,
                # i.e. yield LAG+2) which needs the PSUM banks the division
                # releases -- emitting it later would deadlock the PE queue.
                division = None
                n = 0
                for item in gen:
                    if callable(item):
                        division = item
                        continue
                    n += 1
                    if prev_div is not None and n == div_at:
                        prev_div()
                        prev_div = None
                    if (interleave is not None and n >= interleave_from
                            and (n - interleave_from) % every == 0):
                        for _ in range(per_step):
                            next(interleave, None)
                    if actions and n in actions:
                        actions[n]()
                if prev_div is not None:
                    prev_div()
                return division

            # V'-builds for pair k+1 are hoisted into pair k's chunk loop
            # (DVE is idle mid-pair) so the pair boundary never waits on the
            # stream transposes.
            vps = {}

            def vb_action(bb, h):
                return lambda: vps.__setitem__((bb, h), vbuild(bb, h))

            def vp_of(bb, h):
                return lambda: vps[(bb, h)]

            # batch-1 projections: 8 W-steps paced one-per-chunk into
            # pair(0,0), the remaining 4 into pair(0,1). Pacing the extra PE
            # work evenly keeps the PE ahead of ACT's exp stream in every
            # pair, so no exp backlog accumulates to stall pair boundaries.
            proj_steps = itertools.chain(
                *[proj_tile(tt) for tt in range(NT // 2, NT)]
            )
            vps[(0, 0)] = vbuild(0, 0)
            div00 = run_pair(
                pair_attn(0, 0, vp_of(0, 0)), None,
                itertools.islice(proj_steps, 6), per_step=1,
                interleave_from=2, actions={9: vb_action(0, 1)},
            )
            div01 = run_pair(
                pair_attn(0, 1, vp_of(0, 1)), prev_div=div00,
                interleave=proj_steps, per_step=1, interleave_from=2,
                actions={13: vb_action(1, 0)},
            )
            if phases == "attn2":
                div01()
                nc.sync.dma_start(out=out[:][0:P, :], in_=CX[:, 0:D])
                return
            load_wo()
            op0 = outproj(0)
            div10 = run_pair(
                pair_attn(1, 0, vp_of(1, 0)), prev_div=div01,
                interleave=itertools.islice(op0, 8), per_step=1,
                interleave_from=3, actions={12: vb_action(1, 1)},
            )
            div11 = run_pair(
                pair_attn(1, 1, vp_of(1, 1)), prev_div=div10,
                interleave=op0, per_step=1, interleave_from=2,
            )
            # pipeline the tail: each division q-slice of pair(1,1)
            # unlocks 4 token-groups of outproj(1) immediately
            op1 = outproj(1, ps_pool=pbank, ps_tag="pb")

            def _op1_chase(qs):
                for _ in range(4):
                    next(op1, None)

            div11(after_qs=_op1_chase, rb_engine="act")
            for _ in op1:
                pass


@functools.lru_cache(maxsize=8)
def _get_nc(n_reps: int = 1, phases: str = "full", dyn_reps: bool = False):
    return _build_nc(n_reps, phases, dyn_reps)


def _to_bf16(a):
    import ml_dtypes

    return np.asarray(a, dtype=np.float32).astype(ml_dtypes.bfloat16)


def _shard_inputs(x, Wq, bq, Wk, bk, Wv, bv, Wo, bo):
    x = _to_bf16(x).reshape(T, D)
    # xt4[tt, ki, ko, t] = x[tt*TW + t, ko*P + ki]
    xt4 = np.ascontiguousarray(
        x.reshape(NT, TW, KO, P).transpose(0, 3, 2, 1)
    )
    Wq = _to_bf16(Wq)
    Wk = _to_bf16(Wk)
    Wv = _to_bf16(Wv)
    Wo = _to_bf16(Wo)
    bq = np.asarray(bq, dtype=np.float32)
    bk = np.asarray(bk, dtype=np.float32)
    bv = np.asarray(bv, dtype=np.float32)

    def wtile(W, sl):
        # [ki, ko, f] = W[c*P + f, ko*P + ki]
        return np.ascontiguousarray(
            W[sl, :].reshape(P, KO, P).transpose(2, 1, 0)
        )

    in_maps = []
    for c in range(N_CORES):
        sl = slice(c * P, (c + 1) * P)
        in_maps.append(
            {
                "xt4": xt4,
                "wqT": wtile(Wq, sl),
                "wkT": wtile(Wk, sl),
                "wvT": wtile(Wv, sl),
                "woT": np.ascontiguousarray(Wo[:, sl].T),
                "bq": np.ascontiguousarray(bq[sl].reshape(P, 1)),
                "bk": np.ascontiguousarray(bk[sl].reshape(P, 1)),
                "bv": np.ascontiguousarray(bv[sl].reshape(P, 1)),
            }
        )
    return in_maps


def kernel(x, Wq, bq, Wk, bk, Wv, bv, Wo, bo, **run_kwargs):
    nc = _get_nc()
    in_maps = _shard_inputs(x, Wq, bq, Wk, bk, Wv, bv, Wo, bo)
    last_exc = None
    for _attempt in range(3):
        try:
            res = run_bass_kernel_spmd(
                nc, in_maps, core_ids=list(range(N_CORES)), **run_kwargs
            )
            break
        except Exception as exc:  # transient device errors: retry
            last_exc = exc
            import time as _time

            _time.sleep(3.0)
            # a wedged PJRT client never recovers in-process; force a fresh
            # backend connection so the retry sees recovered devices
            try:
                import jax as _jax

                _jax.clear_caches()
                _jax.extend.backend.clear_backends()
            except Exception:
                pass
    else:
        raise last_exc
    partials = [r["out"] for r in res.results]
    acc = np.add.reduce([np.asarray(p, dtype=np.float32) for p in partials], axis=0)
    acc = acc + np.asarray(bo, dtype=np.float32)[None, :]
    if run_kwargs:
        kernel.last_results = res
    return acc.reshape(B, S, D).astype(np.float32)
